# revision 3
# baseline (speedup 1.0000x reference)
"""Masked dot-product attention (B=16, LQ=LK=2048, D=64) on 8 TRN2 NeuronCores.

Strategy (v2: ACT+DVE/Pool exp split, fp16 P/V)
-----------------------------------------------
out[b] = softmax(mask(Q K^T / 8)) V, keys >= valid_len[b] masked.

Work decomposition identical to the proven baseline: each (batch,
q-half-of-1024) job costs ceil(valid_len/128) k-tiles; jobs are split along k
into segments, sorted, and dealt 8-at-a-time into "slot ranks" across the 8
cores so every core runs the same instruction stream.

Per (slot, k-tile), keys on partitions (S^T layout):
  MM1   S^T[kk, q]  = (K^T chunk).T @ Q^T       (f32r, PSUM)
  EXP   P[kk, q]    = exp(0.125 * S^T)          (fp16, SBUF) -- two engines:
    ACT tiles: scalar-engine table exp (exact).
    DVE tiles: 2-sawtooth-sum approx (max shape err 1.06%, global gain G
      compensated host-side by scaling that tile's V+ones columns by 1/G):
        b1 = trunc_i16(S * 1024*log2e/8 + B1)     (DVE, PSUM->SBUF)
        b2 = b1 + D                               (DVE, int16 4x mode)
        P  = fp16(b1) + fp16(b2)                  (Pool tensor-tensor add)
  MM2   acc[dd|1, q] += ([V chunk | ones]).T @ P  (fp16 in, f32 PSUM accum)
Masking via zeroed V rows + ones-column exactly as the baseline.  Host sums
each job's segments and performs the final divide + transpose.

The exp work splits across engines (ACT ~1038ns, DVE 1519ns + Pool 853ns per
[128,1024] tile) so the ScalarE is no longer the sole softmax bottleneck.
"""

import math
from contextlib import ExitStack

import numpy as np

import concourse.bacc as bacc
import concourse.mybir as mybir
import concourse.tile as tile
import concourse.bass_utils as bass_utils

B, LQ, LK, D = 16, 2048, 2048, 64
N_CORES = 8
KT = 128          # keys per k-tile
QS = 1024         # queries per slot (q-half)
SEG = 8           # max k-tiles per segment
SCALE = 1.0 / math.sqrt(D)

F32 = mybir.dt.float32
F16 = mybir.dt.float16
I16 = mybir.dt.int16
MM_DT = mybir.dt.float32r

# SW2 approx-exp constants (fit: max shape error 0.816%)
C16 = 1024 * 1.4426950408889634 / 8.0
SW_B1 = 15712.0
SW_D = -496.0
SW_GAIN = 2.2533878635239586
RALT = 2          # every RALT'th k-tile uses the DVE/Pool approx path


def dve_tile(kt):
    """Static engine assignment: tile kt of a slot -> approx path?"""
    return (kt % RALT) == 1


def pair_layout(rank_lens, j):
    """Column offsets inside the pair's qk tensor (q | k sections)."""
    na = rank_lens[2 * j]
    qo = 0
    ko = qo + QS
    width = ko + na * KT
    return qo, ko, width


def build_bass(rank_lens, cfg=None):
    cf = {"sp": 3, "ap": 2, "pp": 4, "ep": 3}
    if cfg:
        cf.update(cfg)
    slots = len(rank_lens)
    pairs = slots // 2
    nc = bacc.Bacc("TRN2", target_bir_lowering=False, debug=False)

    widths = [pair_layout(rank_lens, j)[2] for j in range(pairs)]
    vw = [(rank_lens[2 * j] + rank_lens[2 * j + 1]) * (D + 1) for j in range(pairs)]
    pk = [
        nc.dram_tensor(f"pk{j}", [128, widths[j]], MM_DT, kind="ExternalInput").ap()
        for j in range(pairs)
    ]
    pv = [
        nc.dram_tensor(f"pv{j}", [128, vw[j]], F16, kind="ExternalInput").ap()
        for j in range(pairs)
    ]
    out = nc.dram_tensor("out", [slots * (D + 1), QS], F32, kind="ExternalOutput").ap()

    Exp = mybir.ActivationFunctionType.Exp
    Mult = mybir.AluOpType.mult
    Add = mybir.AluOpType.add

    with tile.TileContext(nc) as tc, ExitStack() as ctx:
        inp = ctx.enter_context(tc.tile_pool(name="inp", bufs=1))
        ppool = ctx.enter_context(tc.tile_pool(name="pp", bufs=cf["pp"]))
        bpool = ctx.enter_context(tc.tile_pool(name="bp", bufs=3))
        epool = ctx.enter_context(tc.tile_pool(name="ep", bufs=cf["ep"]))
        spool = ctx.enter_context(tc.tile_pool(name="sp", bufs=cf["sp"], space="PSUM"))
        apool = ctx.enter_context(tc.tile_pool(name="ap", bufs=cf["ap"], space="PSUM"))

        order = cf.get("order") or sorted(range(slots), key=lambda s: rank_lens[s])
        pair_order = sorted(range(pairs), key=lambda j: rank_lens[2 * j])

        qk_t = [None] * pairs
        km_t = [None] * pairs
        kx_t = [None] * pairs
        kx_at = [None] * pairs
        v_t = [None] * pairs
        j0 = pair_order[0]
        dma_seq = [(kind, j) for j in pair_order for kind in ("qk", "v")]
        for kind, j in dma_seq:
            if kind == "qk":
                na, nb = rank_lens[2 * j], rank_lens[2 * j + 1]
                if j in pair_order[:2] and na > nb and nb > 1:
                    w1 = QS + KT
                    qk_t[j] = inp.tile([128, w1], MM_DT, name=f"qk{j}")
                    nc.sync.dma_start(qk_t[j][:], pk[j][:, :w1])
                    km_t[j] = inp.tile([128, (nb - 1) * KT], MM_DT, name=f"km{j}")
                    nc.sync.dma_start(km_t[j][:], pk[j][:, w1 : QS + nb * KT])
                    kx_t[j] = inp.tile([128, (na - nb) * KT], MM_DT, name=f"kx{j}")
                    kx_at[j] = nb
                    nc.sync.dma_start(kx_t[j][:], pk[j][:, QS + nb * KT : widths[j]])
                elif na > nb:
                    wa = QS + nb * KT
                    qk_t[j] = inp.tile([128, wa], MM_DT, name=f"qk{j}")
                    nc.sync.dma_start(qk_t[j][:], pk[j][:, :wa])
                    kx_t[j] = inp.tile([128, (na - nb) * KT], MM_DT, name=f"kx{j}")
                    kx_at[j] = nb
                    nc.sync.dma_start(kx_t[j][:], pk[j][:, wa : widths[j]])
                else:
                    qk_t[j] = inp.tile([128, widths[j]], MM_DT, name=f"qk{j}")
                    nc.sync.dma_start(qk_t[j][:], pk[j][:, : widths[j]])
            else:
                v_t[j] = inp.tile([128, vw[j]], F16, name=f"v{j}")
                nc.sync.dma_start(v_t[j][:], pv[j][:, :])

        for s in order:
            ns = rank_lens[s]
            j = s // 2
            pb = (s % 2) * 64
            qo, ko, _ = pair_layout(rank_lens, j)
            voff = (s % 2) * rank_lens[2 * j] * (D + 1)
            pt = qk_t[j]
            accs = [
                apool.tile([D + 1, 512], F32, name=f"acc{s}_{qq}", tag="acc")
                for qq in range(2)
            ]

            for kt in range(ns):
                s_ps = spool.tile([128, QS], F32, name="s_ps")
                if kx_at[j] is not None and kt >= kx_at[j]:
                    kk = kt - kx_at[j]
                    lhsT = kx_t[j][pb : pb + 64, kk * KT : (kk + 1) * KT]
                elif km_t[j] is not None and kt >= 1:
                    lhsT = km_t[j][pb : pb + 64, (kt - 1) * KT : kt * KT]
                else:
                    lhsT = pt[pb : pb + 64, ko + kt * KT : ko + (kt + 1) * KT]
                for qq in range(2):
                    nc.tensor.matmul(
                        s_ps[:, qq * 512 : (qq + 1) * 512],
                        lhsT,
                        pt[pb : pb + 64, qo + qq * 512 : qo + (qq + 1) * 512],
                        start=True,
                        stop=True,
                    )
                p_t = ppool.tile([128, QS], F16, name="p_t")
                if dve_tile(kt):
                    b1 = bpool.tile([128, QS], I16, name="b1")
                    nc.vector.tensor_scalar(b1[:], s_ps[:], C16, SW_B1, Mult, Add)
                    b2 = bpool.tile([128, QS], I16, name="b2")
                    nc.vector.tensor_scalar(b2[:], b1[:], SW_D, None, Add)
                    nc.gpsimd.tensor_tensor(
                        p_t[:], b1[:].bitcast(F16), b2[:].bitcast(F16), Add
                    )
                else:
                    nc.scalar.activation(p_t[:], s_ps[:], Exp, scale=SCALE)
                w = v_t[j][:, voff + kt * (D + 1) : voff + (kt + 1) * (D + 1)]
                for qq in range(2):
                    nc.tensor.matmul(
                        accs[qq][:, :],
                        w,
                        p_t[:, qq * 512 : (qq + 1) * 512],
                        start=(kt == 0),
                        stop=(kt == ns - 1),
                    )

            for qq in range(2):
                acc_sb = epool.tile([D + 1, 512], F32, name="acc_sb")
                if qq == 0:
                    nc.vector.tensor_copy(acc_sb[:], accs[qq][:])
                else:
                    nc.scalar.copy(acc_sb[:], accs[qq][:])
                nc.sync.dma_start(
                    out[s * (D + 1) : (s + 1) * (D + 1), qq * 512 : (qq + 1) * 512],
                    acc_sb[:],
                )

    nc.compile()
    return nc


def plan_and_pack(queries, keys, values, valid_lens):
    """Split jobs into k-segments, deal into rank slots, gather inputs."""
    q = np.ascontiguousarray(np.asarray(queries, dtype=np.float32))
    k = np.asarray(keys, dtype=np.float32)
    v = np.asarray(values, dtype=np.float32)
    vl = np.asarray(valid_lens, dtype=np.int64)

    nkt = np.maximum(1, -(-vl // KT))

    def make_segs(seg_max):
        segs = []
        for b in range(B):
            n = int(nkt[b])
            m = -(-n // seg_max)
            base, rem = divmod(n, m)
            sizes = [base + 1] * rem + [base] * (m - rem)
            for qh in range(LQ // QS):
                k0 = 0
                for sz in sizes:
                    segs.append((sz, b, qh, k0))
                    k0 += sz
        segs.sort(key=lambda t: (-t[0], t[1], t[2], t[3]))
        return segs

    def cost(segs):
        ls = sorted((s[0] for s in segs), reverse=True)
        while len(ls) % N_CORES:
            ls.append(0)
        slots = len(ls) // N_CORES
        if slots % 2:
            slots += 1
            ls += [0] * N_CORES
        rsum = sum(max(ls[N_CORES * r], 1) for r in range(slots))
        return rsum * 0.90 + slots * 1.2

    seg_best = min(range(5, SEG + 1), key=lambda m: cost(make_segs(m)))
    segs = make_segs(seg_best)
    while len(segs) % N_CORES:
        segs.append(None)
    slots = len(segs) // N_CORES
    if slots % 2:
        segs.extend([None] * N_CORES)
        slots += 1
    rank_lens = []
    for r in range(slots):
        first = segs[N_CORES * r]
        rank_lens.append(first[0] if first is not None else 1)
    pairs = slots // 2

    kT = np.swapaxes(k, 1, 2)
    parts = np.arange(KT)

    in_maps = []
    slot_map = []
    for c in range(N_CORES):
        core_map = {}
        smap = []
        for j in range(pairs):
            qo, ko, width = pair_layout(rank_lens, j)
            na = rank_lens[2 * j]
            pkj = np.zeros((128, width), dtype=np.float32)
            pvj = np.zeros((128, (na + rank_lens[2 * j + 1]) * (D + 1)), dtype=np.float16)
            for i, s in enumerate((2 * j, 2 * j + 1)):
                nr = rank_lens[s]
                seg = segs[N_CORES * s + c]
                if seg is None:
                    smap.append(None)
                    continue
                sz, b, qh, k0 = seg
                pb = i * 64
                smap.append((b, qh, k0))
                pkj[pb : pb + 64, qo : qo + QS] = q[b, qh * QS : (qh + 1) * QS, :].T
                kw = min(nr * KT, LK - k0 * KT)
                pkj[pb : pb + 64, ko : ko + kw] = kT[b, :, k0 * KT : k0 * KT + kw]
                voff = i * na * (D + 1)
                vslab = pvj[:, voff : voff + nr * (D + 1)].reshape(128, nr, D + 1)
                nv = kw // KT
                vs32 = np.zeros((128, nr, D + 1), dtype=np.float32)
                vs32[:, :nv, :D] = (
                    v[b, k0 * KT : k0 * KT + nv * KT, :]
                    .reshape(nv, KT, D)
                    .transpose(1, 0, 2)
                )
                vs32[:, :, D] = 1.0
                kid = (k0 + np.arange(nr))[None, :] * KT + parts[:, None]
                dead = (kid >= vl[b]) | (kid >= (k0 + sz) * KT)
                vs32[dead] = 0.0
                for kt in range(nr):
                    if dve_tile(kt):
                        vs32[:, kt, :] *= 1.0 / SW_GAIN
                vslab[:] = vs32.astype(np.float16)
            core_map[f"pk{j}"] = pkj
            core_map[f"pv{j}"] = pvj
        in_maps.append(core_map)
        slot_map.append(smap)
    return rank_lens, in_maps, slot_map


def scatter_out(results, slot_map):
    acc = {}
    for c in range(N_CORES):
        oc = results[c]["out"]
        for s, seg in enumerate(slot_map[c]):
            if seg is None:
                continue
            b, qh, _ = seg
            blk = oc[s * (D + 1) : (s + 1) * (D + 1), :].astype(np.float64)
            key = (b, qh)
            if key in acc:
                acc[key] += blk
            else:
                acc[key] = blk
    out = np.empty((B, LQ, D), dtype=np.float32)
    for (b, qh), a in acc.items():
        out[b, qh * QS : (qh + 1) * QS, :] = (a[:D, :] / a[D : D + 1, :]).T
    return out


def kernel(queries, keys, values, valid_lens, _run=None):
    rank_lens, in_maps, slot_map = plan_and_pack(queries, keys, values, valid_lens)
    nc = build_bass(rank_lens)
    if _run is not None:
        results = _run(nc, in_maps)
    else:
        import time as _time

        last = None
        for attempt in range(4):
            try:
                results = bass_utils.run_bass_kernel_spmd(
                    nc, in_maps, core_ids=list(range(N_CORES))
                ).results
                break
            except Exception as e:  # noqa: BLE001
                last = e
                _time.sleep(45.0 * (attempt + 1))
        else:
            raise last
    return scatter_out(results, slot_map)


# revision 9
# speedup vs baseline: 1.3727x; 1.3727x over previous
"""Masked dot-product attention (B=16, LQ=LK=2048, D=64) on 8 TRN2 NeuronCores.

Strategy (v3: QS=512 slots, q-partition MM2, ACT+DVE exp split, fp16 P/V)
-------------------------------------------------------------------------
out[b] = softmax(mask(Q K^T / 8)) V, keys >= valid_len[b] masked.

Work decomposition: each (batch, 512-query quarter) job costs
ceil(valid_len/128) k-tiles; jobs are split along k into segments, sorted,
and dealt 8-at-a-time into "slot ranks" across the 8 cores so every core
runs the same instruction stream.  A *unit* is 2 consecutive k-tiles:

  MM1   S^T[kk, u*2+h] = (K^T tile h).T @ Q^T   f32r -> one [128,1024] PSUM
  EXP   P = exp(0.125*S^T) -> fp16 SBUF, alternating engines:
    ACT units: scalar-engine table exp (exact).
    DVE units: 2-sawtooth-sum approx exp (max shape err 1.06%; its global
      gain G is compensated host-side by scaling those tiles' V+ones
      columns by 1/G):
        b1 = trunc_i16(S * 1024*log2e/8 + B1)    (DVE tensor_scalar, 1x)
        b2 = b1 + D                              (DVE int16, 4x mode)
        P  = fp16(b1) + fp16(b2)                 (DVE tensor_tensor, 2x)
  MM2   acc[q, 65*qc+dd|64] += P_chunk.T @ [V|ones]  (fp16 in, f32 PSUM)
    per k-tile h and 128-query chunk qc: out free = 65 -> 8 small matmuls
    per unit at 1 bf16-cycle/row (vs 1024 cycles for the [65,512] form).

acc is ONE [128, 512] PSUM bank per slot (4 q-chunks x 65 cols); the whole
segment accumulates into it (start on first MM2, stop on last).  Masking via
zeroed V rows + ones-column.  Epilogue: one [128,260] copy (ACT/DVE
alternating) + DMA; host sums each job's segments and divides (output is
already [q, d] -- no transpose).

Engine budget per unit (cost model): PE 2x213+8x27=640ns, ACT 1038ns,
DVE-chain 2112ns; with ~1/3 of units on the DVE path all three engines run
~700ns/unit, vs 1038ns/unit for the ACT-only baseline.
"""

import math
from contextlib import ExitStack

import numpy as np

import concourse.bacc as bacc
import concourse.mybir as mybir
import concourse.tile as tile
import concourse.bass_utils as bass_utils

B, LQ, LK, D = 16, 2048, 2048, 64
N_CORES = 8
KT = 128          # keys per k-tile
QS = 512          # queries per slot (q-quarter)
SEG = 8           # max k-tiles per segment
SCALE = 1.0 / math.sqrt(D)

F32 = mybir.dt.float32
F16 = mybir.dt.float16
I16 = mybir.dt.int16
MM_DT = mybir.dt.float32r

# SW2 approx-exp constants (p ~= fp16bits(b1) + fp16bits(b1+D), fit err 1.06%)
C16 = 1024 * 1.4426950408889634 / 8.0
SW_B1 = 15712.0
SW_D = -496.0
SW_GAIN = 2.2533878635239586
DVE_NUM, DVE_DEN = 1, 3   # fraction of k-tiles on the DVE approx path


def dve_unit(s, u):
    """Static engine assignment for k-tile u of slot s (shared with packing)."""
    return ((2 * s + u) % DVE_DEN) >= DVE_DEN - DVE_NUM


def pair_layout(rank_lens, j):
    """Column offsets inside the pair's qk tensor (q | k sections)."""
    na = rank_lens[2 * j]
    qo = 0
    ko = qo + QS
    width = ko + na * KT
    return qo, ko, width


def build_bass(rank_lens, cfg=None):
    cf = {"sp": 6, "pp": 8, "ep": 3, "op3_pool": 2}
    if cfg:
        cf.update(cfg)
    slots = len(rank_lens)
    pairs = slots // 2
    nc = bacc.Bacc("TRN2", target_bir_lowering=False, debug=False)

    widths = [pair_layout(rank_lens, j)[2] for j in range(pairs)]
    vw = [(rank_lens[2 * j] + rank_lens[2 * j + 1]) * (D + 1) for j in range(pairs)]
    pk = [
        nc.dram_tensor(f"pk{j}", [128, widths[j]], MM_DT, kind="ExternalInput").ap()
        for j in range(pairs)
    ]
    pv = [
        nc.dram_tensor(f"pv{j}", [128, vw[j]], F16, kind="ExternalInput").ap()
        for j in range(pairs)
    ]
    out = nc.dram_tensor("out", [slots * 128, 260], F32, kind="ExternalOutput").ap()

    Exp = mybir.ActivationFunctionType.Exp
    Mult = mybir.AluOpType.mult
    Add = mybir.AluOpType.add

    with tile.TileContext(nc) as tc, ExitStack() as ctx:
        inp = ctx.enter_context(tc.tile_pool(name="inp", bufs=1))
        ppool = ctx.enter_context(tc.tile_pool(name="pp", bufs=cf["pp"]))
        bpool = ctx.enter_context(tc.tile_pool(name="bp", bufs=3))
        epool = ctx.enter_context(tc.tile_pool(name="ep", bufs=cf["ep"]))
        spool = ctx.enter_context(tc.tile_pool(name="sp", bufs=cf["sp"], space="PSUM"))
        apool = ctx.enter_context(tc.tile_pool(name="ap", bufs=2, space="PSUM"))

        order = cf.get("order") or sorted(range(slots), key=lambda s: rank_lens[s])
        pair_order = sorted(range(pairs), key=lambda j: rank_lens[2 * j])

        qk_t = [None] * pairs
        km_t = [None] * pairs   # k-tiles [2, nb)
        kx_t = [None] * pairs   # k-tiles [nb, na) (longer slot's overflow)
        kx_at = [None] * pairs
        v_t = [None] * pairs
        dma_seq = [(kind, j) for j in pair_order for kind in ("qk", "v")]
        for kind, j in dma_seq:
            if kind == "qk":
                na, nb = rank_lens[2 * j], rank_lens[2 * j + 1]
                split1 = min(2, nb)
                if j in pair_order[:2] and na > split1:
                    w1 = QS + split1 * KT
                    qk_t[j] = inp.tile([128, w1], MM_DT, name=f"qk{j}")
                    nc.sync.dma_start(qk_t[j][:], pk[j][:, :w1])
                    if nb > split1:
                        km_t[j] = inp.tile([128, (nb - split1) * KT], MM_DT, name=f"km{j}")
                        nc.sync.dma_start(km_t[j][:], pk[j][:, w1 : QS + nb * KT])
                    if na > nb:
                        kx_t[j] = inp.tile([128, (na - nb) * KT], MM_DT, name=f"kx{j}")
                        kx_at[j] = nb
                        nc.sync.dma_start(kx_t[j][:], pk[j][:, QS + nb * KT : widths[j]])
                elif na > nb:
                    wa = QS + nb * KT
                    qk_t[j] = inp.tile([128, wa], MM_DT, name=f"qk{j}")
                    nc.sync.dma_start(qk_t[j][:], pk[j][:, :wa])
                    kx_t[j] = inp.tile([128, (na - nb) * KT], MM_DT, name=f"kx{j}")
                    kx_at[j] = nb
                    nc.sync.dma_start(kx_t[j][:], pk[j][:, wa : widths[j]])
                else:
                    qk_t[j] = inp.tile([128, widths[j]], MM_DT, name=f"qk{j}")
                    nc.sync.dma_start(qk_t[j][:], pk[j][:, : widths[j]])
            else:
                v_t[j] = inp.tile([128, vw[j]], F16, name=f"v{j}")
                nc.sync.dma_start(v_t[j][:], pv[j][:, :])

        def k_lhsT(j, pb, kt):
            if kx_at[j] is not None and kt >= kx_at[j]:
                kk = kt - kx_at[j]
                return kx_t[j][pb : pb + 64, kk * KT : (kk + 1) * KT]
            if km_t[j] is not None and kt >= 2 and qk_t[j].shape[1] <= QS + 2 * KT:
                return km_t[j][pb : pb + 64, (kt - 2) * KT : (kt - 1) * KT]
            ko = QS
            return qk_t[j][pb : pb + 64, ko + kt * KT : ko + (kt + 1) * KT]

        # Flat unit stream across all slots.  Each unit emits MM1 + exp at
        # its turn; its MM2 batch is deferred LAG units (longer for the
        # higher-latency DVE chain) so the in-order PE never head-of-line
        # blocks on a not-yet-computed P tile.
        # Flat per-k-tile stream across all slots: one 1-bank [128,512] PSUM
        # score tile per k-tile gives a 6-deep ring (vs 3 for 2-bank tiles),
        # which is what keeps MM1 from stalling on exp-queue latency.
        op3_pool = cf.get("op3_pool", 0)
        stream = []
        slot_state = {}
        for s in order:
            ns = rank_lens[s]
            slot_state[s] = {"emitted": 0, "ns": ns, "acc": None}
            for kt in range(ns):
                stream.append((s, kt))

        ndve = 0
        for s in order:
            ndve += sum(dve_unit(s, kt) for kt in range(rank_lens[s]))

        dcnt = 0
        for gi, (s, kt) in enumerate(stream):
            ns = rank_lens[s]
            j = s // 2
            pb = (s % 2) * 64
            pt = qk_t[j]
            st = slot_state[s]
            if st["acc"] is None:
                st["acc"] = apool.tile([128, 512], F32, name=f"acc{s}", tag="acc")
            s_ps = spool.tile([128, QS], F32, name="s_ps")
            nc.tensor.matmul(
                s_ps[:, :],
                k_lhsT(j, pb, kt),
                pt[pb : pb + 64, 0:QS],
                start=True,
                stop=True,
            )
            p_t = ppool.tile([128, QS], F16, name="p_t")
            if dve_unit(s, kt):
                b1 = bpool.tile([128, QS], I16, name="b1")
                nc.vector.tensor_scalar(b1[:], s_ps[:], C16, SW_B1, Mult, Add)
                b2 = bpool.tile([128, QS], I16, name="b2")
                nc.vector.tensor_scalar(b2[:], b1[:], SW_D, None, Add)
                dcnt += 1
                eng = nc.gpsimd if (op3_pool and dcnt % op3_pool == 0) else nc.vector
                eng.tensor_tensor(
                    p_t[:], b1[:].bitcast(F16), b2[:].bitcast(F16), Add
                )
            else:
                nc.scalar.activation(p_t[:], s_ps[:], Exp, scale=SCALE)
            voff = (s % 2) * rank_lens[2 * j] * (D + 1)
            wv = v_t[j][:, voff + kt * (D + 1) : voff + (kt + 1) * (D + 1)]
            first = st["emitted"] == 0
            st["emitted"] += 1
            last_batch = st["emitted"] == ns
            for qc in range(4):
                nc.tensor.matmul(
                    st["acc"][:, qc * 65 : qc * 65 + 65],
                    p_t[:, qc * 128 : (qc + 1) * 128],
                    wv,
                    start=(first and qc == 0),
                    stop=(last_batch and qc == 3),
                )
            if last_batch:
                acc_sb = epool.tile([128, 260], F32, name="acc_sb")
                if s % 2 == 0:
                    nc.vector.tensor_copy(acc_sb[:], st["acc"][:, :260])
                else:
                    nc.scalar.copy(acc_sb[:], st["acc"][:, :260])
                nc.sync.dma_start(out[s * 128 : (s + 1) * 128, :], acc_sb[:])

    nc.compile()
    return nc


def plan_and_pack(queries, keys, values, valid_lens):
    """Split jobs into k-segments, deal into rank slots, gather inputs."""
    q = np.ascontiguousarray(np.asarray(queries, dtype=np.float32))
    k = np.asarray(keys, dtype=np.float32)
    v = np.asarray(values, dtype=np.float32)
    vl = np.asarray(valid_lens, dtype=np.int64)

    nkt = np.maximum(1, -(-vl // KT))

    def make_segs(seg_max):
        segs = []  # (len_ktiles, b, qh, k0)
        for b in range(B):
            n = int(nkt[b])
            m = -(-n // seg_max)
            base, rem = divmod(n, m)
            sizes = [base + 1] * rem + [base] * (m - rem)
            for qh in range(LQ // QS):
                k0 = 0
                for sz in sizes:
                    segs.append((sz, b, qh, k0))
                    k0 += sz
        segs.sort(key=lambda t: (-t[0], t[1], t[2], t[3]))
        return segs

    def cost(segs):
        ls = sorted((s[0] for s in segs), reverse=True)
        while len(ls) % N_CORES:
            ls.append(0)
        slots = len(ls) // N_CORES
        if slots % 2:
            slots += 1
            ls += [0] * N_CORES
        usum = sum(-(-max(ls[N_CORES * r], 1) // 2) for r in range(slots))
        return usum * 0.75 + slots * 0.85

    seg_best = min(range(4, SEG + 1), key=lambda m: cost(make_segs(m)))
    segs = make_segs(seg_best)
    while len(segs) % N_CORES:
        segs.append(None)
    slots = len(segs) // N_CORES
    if slots % 2:
        segs.extend([None] * N_CORES)
        slots += 1
    rank_lens = []
    for r in range(slots):
        first = segs[N_CORES * r]
        rank_lens.append(first[0] if first is not None else 1)
    pairs = slots // 2

    kT = np.swapaxes(k, 1, 2)
    parts = np.arange(KT)

    in_maps = []
    slot_map = []
    for c in range(N_CORES):
        core_map = {}
        smap = []
        for j in range(pairs):
            qo, ko, width = pair_layout(rank_lens, j)
            na = rank_lens[2 * j]
            pkj = np.zeros((128, width), dtype=np.float32)
            pvj = np.zeros(
                (128, (na + rank_lens[2 * j + 1]) * (D + 1)), dtype=np.float16
            )
            for i, s in enumerate((2 * j, 2 * j + 1)):
                nr = rank_lens[s]
                seg = segs[N_CORES * s + c]
                if seg is None:
                    smap.append(None)
                    continue
                sz, b, qh, k0 = seg
                pb = i * 64
                smap.append((b, qh, k0))
                pkj[pb : pb + 64, qo : qo + QS] = q[b, qh * QS : (qh + 1) * QS, :].T
                kw = min(nr * KT, LK - k0 * KT)
                pkj[pb : pb + 64, ko : ko + kw] = kT[b, :, k0 * KT : k0 * KT + kw]
                voff = i * na * (D + 1)
                nv = kw // KT
                vs32 = np.zeros((128, nr, D + 1), dtype=np.float32)
                vs32[:, :nv, :D] = (
                    v[b, k0 * KT : k0 * KT + nv * KT, :]
                    .reshape(nv, KT, D)
                    .transpose(1, 0, 2)
                )
                vs32[:, :, D] = 1.0
                kid = (k0 + np.arange(nr))[None, :] * KT + parts[:, None]
                dead = (kid >= vl[b]) | (kid >= (k0 + sz) * KT)
                vs32[dead] = 0.0
                for kt in range(nr):
                    if dve_unit(s, kt):
                        vs32[:, kt, :] *= 1.0 / SW_GAIN
                pvj[:, voff : voff + nr * (D + 1)] = vs32.reshape(
                    128, nr * (D + 1)
                ).astype(np.float16)
            core_map[f"pk{j}"] = pkj
            core_map[f"pv{j}"] = pvj
        in_maps.append(core_map)
        slot_map.append(smap)
    return rank_lens, in_maps, slot_map


def scatter_out(results, slot_map):
    num = {}
    for c in range(N_CORES):
        oc = results[c]["out"]
        for s, seg in enumerate(slot_map[c]):
            if seg is None:
                continue
            b, qh, _ = seg
            blk = oc[s * 128 : (s + 1) * 128, :].astype(np.float64)
            key = (b, qh)
            if key in num:
                num[key] += blk
            else:
                num[key] = blk
    out = np.empty((B, LQ, D), dtype=np.float32)
    for (b, qh), a in num.items():
        a4 = a.reshape(128, 4, 65)
        res = a4[:, :, :D] / a4[:, :, D : D + 1]  # [128q, 4qc, D]
        out[b, qh * QS : (qh + 1) * QS, :] = res.transpose(1, 0, 2).reshape(QS, D)
    return out


def kernel(queries, keys, values, valid_lens, _run=None):
    rank_lens, in_maps, slot_map = plan_and_pack(queries, keys, values, valid_lens)
    nc = build_bass(rank_lens)
    if _run is not None:
        results = _run(nc, in_maps)
    else:
        import time as _time

        last = None
        for attempt in range(4):
            try:
                results = bass_utils.run_bass_kernel_spmd(
                    nc, in_maps, core_ids=list(range(N_CORES))
                ).results
                break
            except Exception as e:  # noqa: BLE001
                last = e
                _time.sleep(45.0 * (attempt + 1))
        else:
            raise last
    return scatter_out(results, slot_map)


# revision 10
# speedup vs baseline: 1.4097x; 1.0270x over previous
"""Masked dot-product attention (B=16, LQ=LK=2048, D=64) on 8 TRN2 NeuronCores.

Strategy (v3: QS=512 slots, q-partition MM2, ACT+DVE exp split, fp16 P/V)
-------------------------------------------------------------------------
out[b] = softmax(mask(Q K^T / 8)) V, keys >= valid_len[b] masked.

Work decomposition: each (batch, 512-query quarter) job costs
ceil(valid_len/128) k-tiles; jobs are split along k into segments, sorted,
and dealt 8-at-a-time into "slot ranks" across the 8 cores so every core
runs the same instruction stream.  A *unit* is 2 consecutive k-tiles:

  MM1   S^T[kk, u*2+h] = (K^T tile h).T @ Q^T   f32r -> one [128,1024] PSUM
  EXP   P = exp(0.125*S^T) -> fp16 SBUF, alternating engines:
    ACT units: scalar-engine table exp (exact).
    DVE units: 2-sawtooth-sum approx exp (max shape err 1.06%; its global
      gain G is compensated host-side by scaling those tiles' V+ones
      columns by 1/G):
        b1 = trunc_i16(S * 1024*log2e/8 + B1)    (DVE tensor_scalar, 1x)
        b2 = b1 + D                              (DVE int16, 4x mode)
        P  = fp16(b1) + fp16(b2)                 (DVE tensor_tensor, 2x)
  MM2   acc[q, 65*qc+dd|64] += P_chunk.T @ [V|ones]  (fp16 in, f32 PSUM)
    per k-tile h and 128-query chunk qc: out free = 65 -> 8 small matmuls
    per unit at 1 bf16-cycle/row (vs 1024 cycles for the [65,512] form).

acc is ONE [128, 512] PSUM bank per slot (4 q-chunks x 65 cols); the whole
segment accumulates into it (start on first MM2, stop on last).  Masking via
zeroed V rows + ones-column.  Epilogue: one [128,260] copy (ACT/DVE
alternating) + DMA; host sums each job's segments and divides (output is
already [q, d] -- no transpose).

Engine budget per unit (cost model): PE 2x213+8x27=640ns, ACT 1038ns,
DVE-chain 2112ns; with ~1/3 of units on the DVE path all three engines run
~700ns/unit, vs 1038ns/unit for the ACT-only baseline.
"""

import math
from contextlib import ExitStack

import numpy as np

import concourse.bacc as bacc
import concourse.mybir as mybir
import concourse.tile as tile
import concourse.bass_utils as bass_utils

B, LQ, LK, D = 16, 2048, 2048, 64
N_CORES = 8
KT = 128          # keys per k-tile
QS = 512          # queries per slot (q-quarter)
SEG = 8           # max k-tiles per segment
SCALE = 1.0 / math.sqrt(D)

F32 = mybir.dt.float32
F16 = mybir.dt.float16
I16 = mybir.dt.int16
MM_DT = mybir.dt.float32r

# SW2 approx-exp constants (p ~= fp16bits(b1) + fp16bits(b1+D), fit err 1.06%)
C16 = 1024 * 1.4426950408889634 / 8.0
SW_B1 = 15712.0
SW_D = -496.0
SW_GAIN = 2.2533878635239586
DVE_NUM, DVE_DEN = 1, 3   # fraction of k-tiles on the DVE approx path


def dve_unit(s, u):
    """Static engine assignment for k-tile u of slot s (shared with packing)."""
    return ((2 * s + u) % DVE_DEN) >= DVE_DEN - DVE_NUM


def pair_layout(rank_lens, j):
    """Column offsets inside the pair's qk tensor (q | k sections)."""
    na = rank_lens[2 * j]
    qo = 0
    ko = qo + QS
    width = ko + na * KT
    return qo, ko, width


def build_bass(rank_lens, cfg=None):
    cf = {"sp": 6, "pp": 10, "ep": 4, "op3_pool": 2}
    if cfg:
        cf.update(cfg)
    slots = len(rank_lens)
    pairs = slots // 2
    nc = bacc.Bacc("TRN2", target_bir_lowering=False, debug=False)

    widths = [pair_layout(rank_lens, j)[2] for j in range(pairs)]
    vw = [(rank_lens[2 * j] + rank_lens[2 * j + 1]) * (D + 1) for j in range(pairs)]
    pk = [
        nc.dram_tensor(f"pk{j}", [128, widths[j]], MM_DT, kind="ExternalInput").ap()
        for j in range(pairs)
    ]
    pv = [
        nc.dram_tensor(f"pv{j}", [128, vw[j]], F16, kind="ExternalInput").ap()
        for j in range(pairs)
    ]
    out = nc.dram_tensor("out", [slots * 128, 260], F32, kind="ExternalOutput").ap()

    Exp = mybir.ActivationFunctionType.Exp
    Mult = mybir.AluOpType.mult
    Add = mybir.AluOpType.add

    with tile.TileContext(nc) as tc, ExitStack() as ctx:
        inp = ctx.enter_context(tc.tile_pool(name="inp", bufs=1))
        ppool = ctx.enter_context(tc.tile_pool(name="pp", bufs=cf["pp"]))
        bpool = ctx.enter_context(tc.tile_pool(name="bp", bufs=3))
        epool = ctx.enter_context(tc.tile_pool(name="ep", bufs=cf["ep"]))
        spool = ctx.enter_context(tc.tile_pool(name="sp", bufs=cf["sp"], space="PSUM"))
        apool = ctx.enter_context(tc.tile_pool(name="ap", bufs=2, space="PSUM"))

        order = cf.get("order") or sorted(range(slots), key=lambda s: rank_lens[s])
        pair_order = sorted(range(pairs), key=lambda j: rank_lens[2 * j])

        qk_t = [None] * pairs
        km_t = [None] * pairs   # k-tiles [2, nb)
        kx_t = [None] * pairs   # k-tiles [nb, na) (longer slot's overflow)
        kx_at = [None] * pairs
        v_t = [None] * pairs
        dma_seq = [(kind, j) for j in pair_order for kind in ("qk", "v")]
        for kind, j in dma_seq:
            if kind == "qk":
                na, nb = rank_lens[2 * j], rank_lens[2 * j + 1]
                split1 = min(2, nb)
                if j in pair_order[:2] and na > split1:
                    w1 = QS + split1 * KT
                    qk_t[j] = inp.tile([128, w1], MM_DT, name=f"qk{j}")
                    nc.sync.dma_start(qk_t[j][:], pk[j][:, :w1])
                    if nb > split1:
                        km_t[j] = inp.tile([128, (nb - split1) * KT], MM_DT, name=f"km{j}")
                        nc.sync.dma_start(km_t[j][:], pk[j][:, w1 : QS + nb * KT])
                    if na > nb:
                        kx_t[j] = inp.tile([128, (na - nb) * KT], MM_DT, name=f"kx{j}")
                        kx_at[j] = nb
                        nc.sync.dma_start(kx_t[j][:], pk[j][:, QS + nb * KT : widths[j]])
                elif na > nb:
                    wa = QS + nb * KT
                    qk_t[j] = inp.tile([128, wa], MM_DT, name=f"qk{j}")
                    nc.sync.dma_start(qk_t[j][:], pk[j][:, :wa])
                    kx_t[j] = inp.tile([128, (na - nb) * KT], MM_DT, name=f"kx{j}")
                    kx_at[j] = nb
                    nc.sync.dma_start(kx_t[j][:], pk[j][:, wa : widths[j]])
                else:
                    qk_t[j] = inp.tile([128, widths[j]], MM_DT, name=f"qk{j}")
                    nc.sync.dma_start(qk_t[j][:], pk[j][:, : widths[j]])
            else:
                v_t[j] = inp.tile([128, vw[j]], F16, name=f"v{j}")
                nc.sync.dma_start(v_t[j][:], pv[j][:, :])

        def k_lhsT(j, pb, kt):
            if kx_at[j] is not None and kt >= kx_at[j]:
                kk = kt - kx_at[j]
                return kx_t[j][pb : pb + 64, kk * KT : (kk + 1) * KT]
            if km_t[j] is not None and kt >= 2 and qk_t[j].shape[1] <= QS + 2 * KT:
                return km_t[j][pb : pb + 64, (kt - 2) * KT : (kt - 1) * KT]
            ko = QS
            return qk_t[j][pb : pb + 64, ko + kt * KT : ko + (kt + 1) * KT]

        # Flat unit stream across all slots.  Each unit emits MM1 + exp at
        # its turn; its MM2 batch is deferred LAG units (longer for the
        # higher-latency DVE chain) so the in-order PE never head-of-line
        # blocks on a not-yet-computed P tile.
        # Flat per-k-tile stream across all slots: one 1-bank [128,512] PSUM
        # score tile per k-tile gives a 6-deep ring (vs 3 for 2-bank tiles),
        # which is what keeps MM1 from stalling on exp-queue latency.
        op3_pool = cf.get("op3_pool", 0)
        stream = []
        slot_state = {}
        for s in order:
            ns = rank_lens[s]
            slot_state[s] = {"emitted": 0, "ns": ns, "acc": None}
            for kt in range(ns):
                stream.append((s, kt))

        ndve = 0
        for s in order:
            ndve += sum(dve_unit(s, kt) for kt in range(rank_lens[s]))

        dcnt = 0
        for gi, (s, kt) in enumerate(stream):
            ns = rank_lens[s]
            j = s // 2
            pb = (s % 2) * 64
            pt = qk_t[j]
            st = slot_state[s]
            if st["acc"] is None:
                st["acc"] = apool.tile([128, 512], F32, name=f"acc{s}", tag="acc")
            s_ps = spool.tile([128, QS], F32, name="s_ps")
            nc.tensor.matmul(
                s_ps[:, :],
                k_lhsT(j, pb, kt),
                pt[pb : pb + 64, 0:QS],
                start=True,
                stop=True,
            )
            p_t = ppool.tile([128, QS], F16, name="p_t")
            if dve_unit(s, kt):
                b1 = bpool.tile([128, QS], I16, name="b1")
                nc.vector.tensor_scalar(b1[:], s_ps[:], C16, SW_B1, Mult, Add)
                b2 = bpool.tile([128, QS], I16, name="b2")
                nc.vector.tensor_scalar(b2[:], b1[:], SW_D, None, Add)
                dcnt += 1
                eng = nc.gpsimd if (op3_pool and dcnt % op3_pool == 0) else nc.vector
                eng.tensor_tensor(
                    p_t[:], b1[:].bitcast(F16), b2[:].bitcast(F16), Add
                )
            else:
                nc.scalar.activation(p_t[:], s_ps[:], Exp, scale=SCALE)
            voff = (s % 2) * rank_lens[2 * j] * (D + 1)
            wv = v_t[j][:, voff + kt * (D + 1) : voff + (kt + 1) * (D + 1)]
            first = st["emitted"] == 0
            st["emitted"] += 1
            last_batch = st["emitted"] == ns
            for qc in range(4):
                nc.tensor.matmul(
                    st["acc"][:, qc * 65 : qc * 65 + 65],
                    p_t[:, qc * 128 : (qc + 1) * 128],
                    wv,
                    start=(first and qc == 0),
                    stop=(last_batch and qc == 3),
                )
            if last_batch:
                acc_sb = epool.tile([128, 260], F32, name="acc_sb")
                if s % 2 == 0:
                    nc.vector.tensor_copy(acc_sb[:], st["acc"][:, :260])
                else:
                    nc.scalar.copy(acc_sb[:], st["acc"][:, :260])
                nc.sync.dma_start(out[s * 128 : (s + 1) * 128, :], acc_sb[:])

    nc.compile()
    return nc


def plan_and_pack(queries, keys, values, valid_lens):
    """Split jobs into k-segments, deal into rank slots, gather inputs."""
    q = np.ascontiguousarray(np.asarray(queries, dtype=np.float32))
    k = np.asarray(keys, dtype=np.float32)
    v = np.asarray(values, dtype=np.float32)
    vl = np.asarray(valid_lens, dtype=np.int64)

    nkt = np.maximum(1, -(-vl // KT))

    def make_segs(seg_max):
        segs = []  # (len_ktiles, b, qh, k0)
        for b in range(B):
            n = int(nkt[b])
            m = -(-n // seg_max)
            base, rem = divmod(n, m)
            sizes = [base + 1] * rem + [base] * (m - rem)
            for qh in range(LQ // QS):
                k0 = 0
                for sz in sizes:
                    segs.append((sz, b, qh, k0))
                    k0 += sz
        segs.sort(key=lambda t: (-t[0], t[1], t[2], t[3]))
        return segs

    def cost(segs):
        ls = sorted((s[0] for s in segs), reverse=True)
        while len(ls) % N_CORES:
            ls.append(0)
        slots = len(ls) // N_CORES
        if slots % 2:
            slots += 1
            ls += [0] * N_CORES
        usum = sum(-(-max(ls[N_CORES * r], 1) // 2) for r in range(slots))
        return usum * 0.75 + slots * 0.85

    seg_best = min(range(4, SEG + 1), key=lambda m: cost(make_segs(m)))
    segs = make_segs(seg_best)
    while len(segs) % N_CORES:
        segs.append(None)
    slots = len(segs) // N_CORES
    if slots % 2:
        segs.extend([None] * N_CORES)
        slots += 1
    rank_lens = []
    for r in range(slots):
        first = segs[N_CORES * r]
        rank_lens.append(first[0] if first is not None else 1)
    pairs = slots // 2

    kT = np.swapaxes(k, 1, 2)
    parts = np.arange(KT)

    in_maps = []
    slot_map = []
    for c in range(N_CORES):
        core_map = {}
        smap = []
        for j in range(pairs):
            qo, ko, width = pair_layout(rank_lens, j)
            na = rank_lens[2 * j]
            pkj = np.zeros((128, width), dtype=np.float32)
            pvj = np.zeros(
                (128, (na + rank_lens[2 * j + 1]) * (D + 1)), dtype=np.float16
            )
            for i, s in enumerate((2 * j, 2 * j + 1)):
                nr = rank_lens[s]
                seg = segs[N_CORES * s + c]
                if seg is None:
                    smap.append(None)
                    continue
                sz, b, qh, k0 = seg
                pb = i * 64
                smap.append((b, qh, k0))
                pkj[pb : pb + 64, qo : qo + QS] = q[b, qh * QS : (qh + 1) * QS, :].T
                kw = min(nr * KT, LK - k0 * KT)
                pkj[pb : pb + 64, ko : ko + kw] = kT[b, :, k0 * KT : k0 * KT + kw]
                voff = i * na * (D + 1)
                nv = kw // KT
                vs32 = np.zeros((128, nr, D + 1), dtype=np.float32)
                vs32[:, :nv, :D] = (
                    v[b, k0 * KT : k0 * KT + nv * KT, :]
                    .reshape(nv, KT, D)
                    .transpose(1, 0, 2)
                )
                vs32[:, :, D] = 1.0
                kid = (k0 + np.arange(nr))[None, :] * KT + parts[:, None]
                dead = (kid >= vl[b]) | (kid >= (k0 + sz) * KT)
                vs32[dead] = 0.0
                for kt in range(nr):
                    if dve_unit(s, kt):
                        vs32[:, kt, :] *= 1.0 / SW_GAIN
                pvj[:, voff : voff + nr * (D + 1)] = vs32.reshape(
                    128, nr * (D + 1)
                ).astype(np.float16)
            core_map[f"pk{j}"] = pkj
            core_map[f"pv{j}"] = pvj
        in_maps.append(core_map)
        slot_map.append(smap)
    return rank_lens, in_maps, slot_map


def scatter_out(results, slot_map):
    num = {}
    for c in range(N_CORES):
        oc = results[c]["out"]
        for s, seg in enumerate(slot_map[c]):
            if seg is None:
                continue
            b, qh, _ = seg
            blk = oc[s * 128 : (s + 1) * 128, :].astype(np.float64)
            key = (b, qh)
            if key in num:
                num[key] += blk
            else:
                num[key] = blk
    out = np.empty((B, LQ, D), dtype=np.float32)
    for (b, qh), a in num.items():
        a4 = a.reshape(128, 4, 65)
        res = a4[:, :, :D] / a4[:, :, D : D + 1]  # [128q, 4qc, D]
        out[b, qh * QS : (qh + 1) * QS, :] = res.transpose(1, 0, 2).reshape(QS, D)
    return out


def kernel(queries, keys, values, valid_lens, _run=None):
    rank_lens, in_maps, slot_map = plan_and_pack(queries, keys, values, valid_lens)
    nc = build_bass(rank_lens)
    if _run is not None:
        results = _run(nc, in_maps)
    else:
        import time as _time

        last = None
        for attempt in range(4):
            try:
                results = bass_utils.run_bass_kernel_spmd(
                    nc, in_maps, core_ids=list(range(N_CORES))
                ).results
                break
            except Exception as e:  # noqa: BLE001
                last = e
                _time.sleep(45.0 * (attempt + 1))
        else:
            raise last
    return scatter_out(results, slot_map)


# revision 13
# speedup vs baseline: 1.4848x; 1.0533x over previous
"""Masked dot-product attention (B=16, LQ=LK=2048, D=64) on 8 TRN2 NeuronCores.

Strategy (v3: QS=512 slots, q-partition MM2, ACT+DVE exp split, fp16 P/V)
-------------------------------------------------------------------------
out[b] = softmax(mask(Q K^T / 8)) V, keys >= valid_len[b] masked.

Work decomposition: each (batch, 512-query quarter) job costs
ceil(valid_len/128) k-tiles; jobs are split along k into segments, sorted,
and dealt 8-at-a-time into "slot ranks" across the 8 cores so every core
runs the same instruction stream.  A *unit* is 2 consecutive k-tiles:

  MM1   S^T[kk, u*2+h] = (K^T tile h).T @ Q^T   f32r -> one [128,1024] PSUM
  EXP   P = exp(0.125*S^T) -> fp16 SBUF, alternating engines:
    ACT units: scalar-engine table exp (exact).
    DVE units: 2-sawtooth-sum approx exp (max shape err 1.06%; its global
      gain G is compensated host-side by scaling those tiles' V+ones
      columns by 1/G):
        b1 = trunc_i16(S * 1024*log2e/8 + B1)    (DVE tensor_scalar, 1x)
        b2 = b1 + D                              (DVE int16, 4x mode)
        P  = fp16(b1) + fp16(b2)                 (DVE tensor_tensor, 2x)
  MM2   acc[q, 65*qc+dd|64] += P_chunk.T @ [V|ones]  (fp16 in, f32 PSUM)
    per k-tile h and 128-query chunk qc: out free = 65 -> 8 small matmuls
    per unit at 1 bf16-cycle/row (vs 1024 cycles for the [65,512] form).

acc is ONE [128, 512] PSUM bank per slot (4 q-chunks x 65 cols); the whole
segment accumulates into it (start on first MM2, stop on last).  Masking via
zeroed V rows + ones-column.  Epilogue: one [128,260] copy (ACT/DVE
alternating) + DMA; host sums each job's segments and divides (output is
already [q, d] -- no transpose).

Engine budget per unit (cost model): PE 2x213+8x27=640ns, ACT 1038ns,
DVE-chain 2112ns; with ~1/3 of units on the DVE path all three engines run
~700ns/unit, vs 1038ns/unit for the ACT-only baseline.
"""

import math
from contextlib import ExitStack

import numpy as np

import concourse.bacc as bacc
import concourse.mybir as mybir
import concourse.tile as tile
import concourse.bass_utils as bass_utils

B, LQ, LK, D = 16, 2048, 2048, 64
N_CORES = 8
KT = 128          # keys per k-tile
QS = 512          # queries per slot (q-quarter)
SEG = 16          # max k-tiles per segment (16 = whole jobs)
SCALE = 1.0 / math.sqrt(D)

F32 = mybir.dt.float32
F16 = mybir.dt.float16
I16 = mybir.dt.int16
MM_DT = mybir.dt.float32r

# SW2 approx-exp constants (p ~= fp16bits(b1) + fp16bits(b1+D), fit err 1.06%)
C16 = 1024 * 1.4426950408889634 / 8.0
SW_B1 = 15712.0
SW_D = -496.0
SW_GAIN = 2.2533878635239586
def dve_unit(s, u):
    """Static engine assignment for k-tile u of slot s (shared with packing)."""
    return ((2 * s + u) % 3) == 1


def pair_layout(rank_lens, j):
    """Column offsets inside the pair's qk tensor (q | k sections)."""
    na = rank_lens[2 * j]
    qo = 0
    ko = qo + QS
    width = ko + na * KT
    return qo, ko, width


def build_bass(rank_lens, cfg=None):
    cf = {"sp": 6, "pp": 10, "ep": 4, "op3_pool": 2, "copy_act": 0}
    if cfg:
        cf.update(cfg)
    slots = len(rank_lens)
    pairs = slots // 2
    nc = bacc.Bacc("TRN2", target_bir_lowering=False, debug=False)

    widths = [pair_layout(rank_lens, j)[2] for j in range(pairs)]
    vw = [(rank_lens[2 * j] + rank_lens[2 * j + 1]) * (D + 1) for j in range(pairs)]
    pk = [
        nc.dram_tensor(f"pk{j}", [128, widths[j]], MM_DT, kind="ExternalInput").ap()
        for j in range(pairs)
    ]
    pv = [
        nc.dram_tensor(f"pv{j}", [128, vw[j]], F16, kind="ExternalInput").ap()
        for j in range(pairs)
    ]
    out = nc.dram_tensor("out", [slots * 128, 260], F32, kind="ExternalOutput").ap()

    Exp = mybir.ActivationFunctionType.Exp
    Mult = mybir.AluOpType.mult
    Add = mybir.AluOpType.add

    with tile.TileContext(nc) as tc, ExitStack() as ctx:
        inp = ctx.enter_context(tc.tile_pool(name="inp", bufs=1))
        ppool = ctx.enter_context(tc.tile_pool(name="pp", bufs=cf["pp"]))
        bpool = ctx.enter_context(tc.tile_pool(name="bp", bufs=3))
        epool = ctx.enter_context(tc.tile_pool(name="ep", bufs=cf["ep"]))
        spool = ctx.enter_context(tc.tile_pool(name="sp", bufs=cf["sp"], space="PSUM"))
        apool = ctx.enter_context(tc.tile_pool(name="ap", bufs=2, space="PSUM"))

        order = cf.get("order") or sorted(range(slots), key=lambda s: rank_lens[s])
        pair_order = sorted(range(pairs), key=lambda j: rank_lens[2 * j])

        qk_t = [None] * pairs
        km_t = [None] * pairs   # k-tiles [2, nb)
        kx_t = [None] * pairs   # k-tiles [nb, na) (longer slot's overflow)
        kx_at = [None] * pairs
        v_t = [None] * pairs
        dma_seq = [(kind, j) for j in pair_order for kind in ("qk", "v")]
        for kind, j in dma_seq:
            if kind == "qk":
                na, nb = rank_lens[2 * j], rank_lens[2 * j + 1]
                split1 = min(2, nb)
                if j in pair_order[:2] and na > split1:
                    w1 = QS + split1 * KT
                    qk_t[j] = inp.tile([128, w1], MM_DT, name=f"qk{j}")
                    nc.sync.dma_start(qk_t[j][:], pk[j][:, :w1])
                    if nb > split1:
                        km_t[j] = inp.tile([128, (nb - split1) * KT], MM_DT, name=f"km{j}")
                        nc.sync.dma_start(km_t[j][:], pk[j][:, w1 : QS + nb * KT])
                    if na > nb:
                        kx_t[j] = inp.tile([128, (na - nb) * KT], MM_DT, name=f"kx{j}")
                        kx_at[j] = nb
                        nc.sync.dma_start(kx_t[j][:], pk[j][:, QS + nb * KT : widths[j]])
                elif na > nb:
                    wa = QS + nb * KT
                    qk_t[j] = inp.tile([128, wa], MM_DT, name=f"qk{j}")
                    nc.sync.dma_start(qk_t[j][:], pk[j][:, :wa])
                    kx_t[j] = inp.tile([128, (na - nb) * KT], MM_DT, name=f"kx{j}")
                    kx_at[j] = nb
                    nc.sync.dma_start(kx_t[j][:], pk[j][:, wa : widths[j]])
                else:
                    qk_t[j] = inp.tile([128, widths[j]], MM_DT, name=f"qk{j}")
                    nc.sync.dma_start(qk_t[j][:], pk[j][:, : widths[j]])
            else:
                v_t[j] = inp.tile([128, vw[j]], F16, name=f"v{j}")
                nc.sync.dma_start(v_t[j][:], pv[j][:, :])

        def k_lhsT(j, pb, kt):
            if kx_at[j] is not None and kt >= kx_at[j]:
                kk = kt - kx_at[j]
                return kx_t[j][pb : pb + 64, kk * KT : (kk + 1) * KT]
            if km_t[j] is not None and kt >= 2 and qk_t[j].shape[1] <= QS + 2 * KT:
                return km_t[j][pb : pb + 64, (kt - 2) * KT : (kt - 1) * KT]
            ko = QS
            return qk_t[j][pb : pb + 64, ko + kt * KT : ko + (kt + 1) * KT]

        # Flat unit stream across all slots.  Each unit emits MM1 + exp at
        # its turn; its MM2 batch is deferred LAG units (longer for the
        # higher-latency DVE chain) so the in-order PE never head-of-line
        # blocks on a not-yet-computed P tile.
        # Flat per-k-tile stream across all slots: one 1-bank [128,512] PSUM
        # score tile per k-tile gives a 6-deep ring (vs 3 for 2-bank tiles),
        # which is what keeps MM1 from stalling on exp-queue latency.
        op3_pool = cf.get("op3_pool", 0)
        stream = []
        slot_state = {}
        for s in order:
            ns = rank_lens[s]
            slot_state[s] = {"emitted": 0, "ns": ns, "acc": None}
            for kt in range(ns):
                stream.append((s, kt))

        ndve = 0
        for s in order:
            ndve += sum(dve_unit(s, kt) for kt in range(rank_lens[s]))

        dcnt = 0
        for gi, (s, kt) in enumerate(stream):
            ns = rank_lens[s]
            j = s // 2
            pb = (s % 2) * 64
            pt = qk_t[j]
            st = slot_state[s]
            if st["acc"] is None:
                st["acc"] = apool.tile([128, 512], F32, name=f"acc{s}", tag="acc")
            s_ps = spool.tile([128, QS], F32, name="s_ps")
            nc.tensor.matmul(
                s_ps[:, :],
                k_lhsT(j, pb, kt),
                pt[pb : pb + 64, 0:QS],
                start=True,
                stop=True,
            )
            p_t = ppool.tile([128, QS], F16, name="p_t")
            if dve_unit(s, kt):
                b1 = bpool.tile([128, QS], I16, name="b1")
                nc.vector.tensor_scalar(b1[:], s_ps[:], C16, SW_B1, Mult, Add)
                b2 = bpool.tile([128, QS], I16, name="b2")
                nc.vector.tensor_scalar(b2[:], b1[:], SW_D, None, Add)
                dcnt += 1
                eng = nc.gpsimd if (op3_pool and dcnt % op3_pool == 0) else nc.vector
                eng.tensor_tensor(
                    p_t[:], b1[:].bitcast(F16), b2[:].bitcast(F16), Add
                )
            else:
                nc.scalar.activation(p_t[:], s_ps[:], Exp, scale=SCALE)
            voff = (s % 2) * rank_lens[2 * j] * (D + 1)
            wv = v_t[j][:, voff + kt * (D + 1) : voff + (kt + 1) * (D + 1)]
            first = st["emitted"] == 0
            st["emitted"] += 1
            last_batch = st["emitted"] == ns
            for qc in range(4):
                nc.tensor.matmul(
                    st["acc"][:, qc * 65 : qc * 65 + 65],
                    p_t[:, qc * 128 : (qc + 1) * 128],
                    wv,
                    start=(first and qc == 0),
                    stop=(last_batch and qc == 3),
                )
            if last_batch:
                acc_sb = epool.tile([128, 260], F32, name="acc_sb")
                ca = cf.get("copy_act", 2)
                if ca and s % ca == 1:
                    nc.scalar.copy(acc_sb[:], st["acc"][:, :260])
                else:
                    nc.vector.tensor_copy(acc_sb[:], st["acc"][:, :260])
                nc.sync.dma_start(out[s * 128 : (s + 1) * 128, :], acc_sb[:])

    nc.compile()
    return nc


def plan_and_pack(queries, keys, values, valid_lens):
    """Split jobs into k-segments, deal into rank slots, gather inputs."""
    q = np.ascontiguousarray(np.asarray(queries, dtype=np.float32))
    k = np.asarray(keys, dtype=np.float32)
    v = np.asarray(values, dtype=np.float32)
    vl = np.asarray(valid_lens, dtype=np.int64)

    nkt = np.maximum(1, -(-vl // KT))

    def make_segs(seg_max):
        segs = []  # (len_ktiles, b, qh, k0)
        for b in range(B):
            n = int(nkt[b])
            m = -(-n // seg_max)
            base, rem = divmod(n, m)
            sizes = [base + 1] * rem + [base] * (m - rem)
            for qh in range(LQ // QS):
                k0 = 0
                for sz in sizes:
                    segs.append((sz, b, qh, k0))
                    k0 += sz
        segs.sort(key=lambda t: (-t[0], t[1], t[2], t[3]))
        return segs

    def cost(segs):
        ls = sorted((s[0] for s in segs), reverse=True)
        while len(ls) % N_CORES:
            ls.append(0)
        slots = len(ls) // N_CORES
        if slots % 2:
            slots += 1
            ls += [0] * N_CORES
        rsum = sum(max(ls[N_CORES * r], 1) for r in range(slots))
        return rsum * 0.62 + slots * 0.8

    seg_best = min(range(4, SEG + 1), key=lambda m: cost(make_segs(m)))
    segs = make_segs(seg_best)
    while len(segs) % N_CORES:
        segs.append(None)
    slots = len(segs) // N_CORES
    if slots % 2:
        segs.extend([None] * N_CORES)
        slots += 1
    rank_lens = []
    for r in range(slots):
        first = segs[N_CORES * r]
        rank_lens.append(first[0] if first is not None else 1)
    pairs = slots // 2

    kT = np.swapaxes(k, 1, 2)
    parts = np.arange(KT)

    in_maps = []
    slot_map = []
    for c in range(N_CORES):
        core_map = {}
        smap = []
        for j in range(pairs):
            qo, ko, width = pair_layout(rank_lens, j)
            na = rank_lens[2 * j]
            pkj = np.zeros((128, width), dtype=np.float32)
            pvj = np.zeros(
                (128, (na + rank_lens[2 * j + 1]) * (D + 1)), dtype=np.float16
            )
            for i, s in enumerate((2 * j, 2 * j + 1)):
                nr = rank_lens[s]
                seg = segs[N_CORES * s + c]
                if seg is None:
                    smap.append(None)
                    continue
                sz, b, qh, k0 = seg
                pb = i * 64
                smap.append((b, qh, k0))
                pkj[pb : pb + 64, qo : qo + QS] = q[b, qh * QS : (qh + 1) * QS, :].T
                kw = min(nr * KT, LK - k0 * KT)
                pkj[pb : pb + 64, ko : ko + kw] = kT[b, :, k0 * KT : k0 * KT + kw]
                voff = i * na * (D + 1)
                nv = kw // KT
                vs32 = np.zeros((128, nr, D + 1), dtype=np.float32)
                vs32[:, :nv, :D] = (
                    v[b, k0 * KT : k0 * KT + nv * KT, :]
                    .reshape(nv, KT, D)
                    .transpose(1, 0, 2)
                )
                vs32[:, :, D] = 1.0
                kid = (k0 + np.arange(nr))[None, :] * KT + parts[:, None]
                dead = (kid >= vl[b]) | (kid >= (k0 + sz) * KT)
                vs32[dead] = 0.0
                for kt in range(nr):
                    if dve_unit(s, kt):
                        vs32[:, kt, :] *= 1.0 / SW_GAIN
                pvj[:, voff : voff + nr * (D + 1)] = vs32.reshape(
                    128, nr * (D + 1)
                ).astype(np.float16)
            core_map[f"pk{j}"] = pkj
            core_map[f"pv{j}"] = pvj
        in_maps.append(core_map)
        slot_map.append(smap)
    return rank_lens, in_maps, slot_map


def scatter_out(results, slot_map):
    num = {}
    for c in range(N_CORES):
        oc = results[c]["out"]
        for s, seg in enumerate(slot_map[c]):
            if seg is None:
                continue
            b, qh, _ = seg
            blk = oc[s * 128 : (s + 1) * 128, :].astype(np.float64)
            key = (b, qh)
            if key in num:
                num[key] += blk
            else:
                num[key] = blk
    out = np.empty((B, LQ, D), dtype=np.float32)
    for (b, qh), a in num.items():
        a4 = a.reshape(128, 4, 65)
        res = a4[:, :, :D] / a4[:, :, D : D + 1]  # [128q, 4qc, D]
        out[b, qh * QS : (qh + 1) * QS, :] = res.transpose(1, 0, 2).reshape(QS, D)
    return out


def kernel(queries, keys, values, valid_lens, _run=None):
    rank_lens, in_maps, slot_map = plan_and_pack(queries, keys, values, valid_lens)
    nc = build_bass(rank_lens)
    if _run is not None:
        results = _run(nc, in_maps)
    else:
        import time as _time

        last = None
        for attempt in range(4):
            try:
                results = bass_utils.run_bass_kernel_spmd(
                    nc, in_maps, core_ids=list(range(N_CORES))
                ).results
                break
            except Exception as e:  # noqa: BLE001
                last = e
                _time.sleep(45.0 * (attempt + 1))
        else:
            raise last
    return scatter_out(results, slot_map)


# revision 14
# speedup vs baseline: 1.5065x; 1.0146x over previous
"""Masked dot-product attention (B=16, LQ=LK=2048, D=64) on 8 TRN2 NeuronCores.

Strategy (v3: QS=512 slots, q-partition MM2, ACT+DVE exp split, fp16 P/V)
-------------------------------------------------------------------------
out[b] = softmax(mask(Q K^T / 8)) V, keys >= valid_len[b] masked.

Work decomposition: each (batch, 512-query quarter) job costs
ceil(valid_len/128) k-tiles; jobs are split along k into segments, sorted,
and dealt 8-at-a-time into "slot ranks" across the 8 cores so every core
runs the same instruction stream.  A *unit* is 2 consecutive k-tiles:

  MM1   S^T[kk, u*2+h] = (K^T tile h).T @ Q^T   f32r -> one [128,1024] PSUM
  EXP   P = exp(0.125*S^T) -> fp16 SBUF, alternating engines:
    ACT units: scalar-engine table exp (exact).
    DVE units: 2-sawtooth-sum approx exp (max shape err 1.06%; its global
      gain G is compensated host-side by scaling those tiles' V+ones
      columns by 1/G):
        b1 = trunc_i16(S * 1024*log2e/8 + B1)    (DVE tensor_scalar, 1x)
        b2 = b1 + D                              (DVE int16, 4x mode)
        P  = fp16(b1) + fp16(b2)                 (DVE tensor_tensor, 2x)
  MM2   acc[q, 65*qc+dd|64] += P_chunk.T @ [V|ones]  (fp16 in, f32 PSUM)
    per k-tile h and 128-query chunk qc: out free = 65 -> 8 small matmuls
    per unit at 1 bf16-cycle/row (vs 1024 cycles for the [65,512] form).

acc is ONE [128, 512] PSUM bank per slot (4 q-chunks x 65 cols); the whole
segment accumulates into it (start on first MM2, stop on last).  Masking via
zeroed V rows + ones-column.  Epilogue: one [128,260] copy (ACT/DVE
alternating) + DMA; host sums each job's segments and divides (output is
already [q, d] -- no transpose).

Engine budget per unit (cost model): PE 2x213+8x27=640ns, ACT 1038ns,
DVE-chain 2112ns; with ~1/3 of units on the DVE path all three engines run
~700ns/unit, vs 1038ns/unit for the ACT-only baseline.
"""

import math
from contextlib import ExitStack

import numpy as np

import concourse.bacc as bacc
import concourse.mybir as mybir
import concourse.tile as tile
import concourse.bass_utils as bass_utils

B, LQ, LK, D = 16, 2048, 2048, 64
N_CORES = 8
KT = 128          # keys per k-tile
QS = 512          # queries per slot (q-quarter)
SEG = 16          # max k-tiles per segment (16 = whole jobs)
SCALE = 1.0 / math.sqrt(D)

F32 = mybir.dt.float32
F16 = mybir.dt.float16
I16 = mybir.dt.int16
MM_DT = mybir.dt.float32r

# SW2 approx-exp constants (p ~= fp16bits(b1) + fp16bits(b1+D), fit err 1.06%)
C16 = 1024 * 1.4426950408889634 / 8.0
SW_B1 = 15712.0
SW_D = -496.0
SW_GAIN = 2.2533878635239586
def dve_unit(s, u):
    """Static engine assignment for k-tile u of slot s (shared with packing)."""
    return ((2 * s + u) % 3) == 1


def pair_layout(rank_lens, j):
    """Column offsets inside the pair's qk tensor (q | k sections)."""
    na = rank_lens[2 * j]
    qo = 0
    ko = qo + QS
    width = ko + na * KT
    return qo, ko, width


def build_bass(rank_lens, cfg=None):
    cf = {"sp": 6, "pp": 10, "ep": 4, "op3_pool": 2, "copy_act": 0}
    if cfg:
        cf.update(cfg)
    slots = len(rank_lens)
    pairs = slots // 2
    nc = bacc.Bacc("TRN2", target_bir_lowering=False, debug=False)

    widths = [pair_layout(rank_lens, j)[2] for j in range(pairs)]
    vw = [(rank_lens[2 * j] + rank_lens[2 * j + 1]) * (D + 1) for j in range(pairs)]
    pk = [
        nc.dram_tensor(f"pk{j}", [128, widths[j]], MM_DT, kind="ExternalInput").ap()
        for j in range(pairs)
    ]
    pv = [
        nc.dram_tensor(f"pv{j}", [128, vw[j]], F16, kind="ExternalInput").ap()
        for j in range(pairs)
    ]
    out = nc.dram_tensor("out", [slots * 128, 260], F32, kind="ExternalOutput").ap()

    Exp = mybir.ActivationFunctionType.Exp
    Mult = mybir.AluOpType.mult
    Add = mybir.AluOpType.add

    with tile.TileContext(nc) as tc, ExitStack() as ctx:
        inp = ctx.enter_context(tc.tile_pool(name="inp", bufs=1))
        ppool = ctx.enter_context(tc.tile_pool(name="pp", bufs=cf["pp"]))
        bpool = ctx.enter_context(tc.tile_pool(name="bp", bufs=3))
        epool = ctx.enter_context(tc.tile_pool(name="ep", bufs=cf["ep"]))
        spool = ctx.enter_context(tc.tile_pool(name="sp", bufs=cf["sp"], space="PSUM"))
        apool = ctx.enter_context(tc.tile_pool(name="ap", bufs=2, space="PSUM"))

        asc = sorted(range(slots), key=lambda s: rank_lens[s])
        # shortest slots first (fast DMA startup); longest slot second-to-last
        # so the very last slot's epilogue tail is short
        order = cf.get("order") or (asc[:-2] + [asc[-1], asc[-2]] if slots > 2 else asc)
        pair_order = sorted(range(pairs), key=lambda j: rank_lens[2 * j])

        qk_t = [None] * pairs
        km_t = [None] * pairs   # k-tiles [2, nb)
        kx_t = [None] * pairs   # k-tiles [nb, na) (longer slot's overflow)
        kx_at = [None] * pairs
        v_t = [None] * pairs
        dma_seq = [(kind, j) for j in pair_order for kind in ("qk", "v")]
        for kind, j in dma_seq:
            if kind == "qk":
                na, nb = rank_lens[2 * j], rank_lens[2 * j + 1]
                split1 = min(2, nb)
                if j in pair_order[:2] and na > split1:
                    w1 = QS + split1 * KT
                    qk_t[j] = inp.tile([128, w1], MM_DT, name=f"qk{j}")
                    nc.sync.dma_start(qk_t[j][:], pk[j][:, :w1])
                    if nb > split1:
                        km_t[j] = inp.tile([128, (nb - split1) * KT], MM_DT, name=f"km{j}")
                        nc.sync.dma_start(km_t[j][:], pk[j][:, w1 : QS + nb * KT])
                    if na > nb:
                        kx_t[j] = inp.tile([128, (na - nb) * KT], MM_DT, name=f"kx{j}")
                        kx_at[j] = nb
                        nc.sync.dma_start(kx_t[j][:], pk[j][:, QS + nb * KT : widths[j]])
                elif na > nb:
                    wa = QS + nb * KT
                    qk_t[j] = inp.tile([128, wa], MM_DT, name=f"qk{j}")
                    nc.sync.dma_start(qk_t[j][:], pk[j][:, :wa])
                    kx_t[j] = inp.tile([128, (na - nb) * KT], MM_DT, name=f"kx{j}")
                    kx_at[j] = nb
                    nc.sync.dma_start(kx_t[j][:], pk[j][:, wa : widths[j]])
                else:
                    qk_t[j] = inp.tile([128, widths[j]], MM_DT, name=f"qk{j}")
                    nc.sync.dma_start(qk_t[j][:], pk[j][:, : widths[j]])
            else:
                v_t[j] = inp.tile([128, vw[j]], F16, name=f"v{j}")
                nc.sync.dma_start(v_t[j][:], pv[j][:, :])

        def k_lhsT(j, pb, kt):
            if kx_at[j] is not None and kt >= kx_at[j]:
                kk = kt - kx_at[j]
                return kx_t[j][pb : pb + 64, kk * KT : (kk + 1) * KT]
            if km_t[j] is not None and kt >= 2 and qk_t[j].shape[1] <= QS + 2 * KT:
                return km_t[j][pb : pb + 64, (kt - 2) * KT : (kt - 1) * KT]
            ko = QS
            return qk_t[j][pb : pb + 64, ko + kt * KT : ko + (kt + 1) * KT]

        # Flat unit stream across all slots.  Each unit emits MM1 + exp at
        # its turn; its MM2 batch is deferred LAG units (longer for the
        # higher-latency DVE chain) so the in-order PE never head-of-line
        # blocks on a not-yet-computed P tile.
        # Flat per-k-tile stream across all slots: one 1-bank [128,512] PSUM
        # score tile per k-tile gives a 6-deep ring (vs 3 for 2-bank tiles),
        # which is what keeps MM1 from stalling on exp-queue latency.
        op3_pool = cf.get("op3_pool", 0)
        stream = []
        slot_state = {}
        for s in order:
            ns = rank_lens[s]
            slot_state[s] = {"emitted": 0, "ns": ns, "acc": None}
            for kt in range(ns):
                stream.append((s, kt))

        ndve = 0
        for s in order:
            ndve += sum(dve_unit(s, kt) for kt in range(rank_lens[s]))

        dcnt = 0
        for gi, (s, kt) in enumerate(stream):
            ns = rank_lens[s]
            j = s // 2
            pb = (s % 2) * 64
            pt = qk_t[j]
            st = slot_state[s]
            if st["acc"] is None:
                st["acc"] = apool.tile([128, 512], F32, name=f"acc{s}", tag="acc")
            s_ps = spool.tile([128, QS], F32, name="s_ps")
            nc.tensor.matmul(
                s_ps[:, :],
                k_lhsT(j, pb, kt),
                pt[pb : pb + 64, 0:QS],
                start=True,
                stop=True,
            )
            p_t = ppool.tile([128, QS], F16, name="p_t")
            if dve_unit(s, kt):
                b1 = bpool.tile([128, QS], I16, name="b1")
                nc.vector.tensor_scalar(b1[:], s_ps[:], C16, SW_B1, Mult, Add)
                b2 = bpool.tile([128, QS], I16, name="b2")
                nc.vector.tensor_scalar(b2[:], b1[:], SW_D, None, Add)
                dcnt += 1
                eng = nc.gpsimd if (op3_pool and dcnt % op3_pool == 0) else nc.vector
                eng.tensor_tensor(
                    p_t[:], b1[:].bitcast(F16), b2[:].bitcast(F16), Add
                )
            else:
                nc.scalar.activation(p_t[:], s_ps[:], Exp, scale=SCALE)
            voff = (s % 2) * rank_lens[2 * j] * (D + 1)
            wv = v_t[j][:, voff + kt * (D + 1) : voff + (kt + 1) * (D + 1)]
            first = st["emitted"] == 0
            st["emitted"] += 1
            last_batch = st["emitted"] == ns
            for qc in range(4):
                nc.tensor.matmul(
                    st["acc"][:, qc * 65 : qc * 65 + 65],
                    p_t[:, qc * 128 : (qc + 1) * 128],
                    wv,
                    start=(first and qc == 0),
                    stop=(last_batch and qc == 3),
                )
            if last_batch:
                acc_sb = epool.tile([128, 260], F32, name="acc_sb")
                ca = cf.get("copy_act", 2)
                if ca and s % ca == 1:
                    nc.scalar.copy(acc_sb[:], st["acc"][:, :260])
                else:
                    nc.vector.tensor_copy(acc_sb[:], st["acc"][:, :260])
                nc.sync.dma_start(out[s * 128 : (s + 1) * 128, :], acc_sb[:])

    nc.compile()
    return nc


def plan_and_pack(queries, keys, values, valid_lens):
    """Split jobs into k-segments, deal into rank slots, gather inputs."""
    q = np.ascontiguousarray(np.asarray(queries, dtype=np.float32))
    k = np.asarray(keys, dtype=np.float32)
    v = np.asarray(values, dtype=np.float32)
    vl = np.asarray(valid_lens, dtype=np.int64)

    nkt = np.maximum(1, -(-vl // KT))

    def make_segs(seg_max):
        segs = []  # (len_ktiles, b, qh, k0)
        for b in range(B):
            n = int(nkt[b])
            m = -(-n // seg_max)
            base, rem = divmod(n, m)
            sizes = [base + 1] * rem + [base] * (m - rem)
            for qh in range(LQ // QS):
                k0 = 0
                for sz in sizes:
                    segs.append((sz, b, qh, k0))
                    k0 += sz
        segs.sort(key=lambda t: (-t[0], t[1], t[2], t[3]))
        return segs

    def cost(segs):
        ls = sorted((s[0] for s in segs), reverse=True)
        while len(ls) % N_CORES:
            ls.append(0)
        slots = len(ls) // N_CORES
        if slots % 2:
            slots += 1
            ls += [0] * N_CORES
        rsum = sum(max(ls[N_CORES * r], 1) for r in range(slots))
        return rsum * 0.62 + slots * 0.8

    seg_best = min(range(4, SEG + 1), key=lambda m: cost(make_segs(m)))
    segs = make_segs(seg_best)
    while len(segs) % N_CORES:
        segs.append(None)
    slots = len(segs) // N_CORES
    if slots % 2:
        segs.extend([None] * N_CORES)
        slots += 1
    rank_lens = []
    for r in range(slots):
        first = segs[N_CORES * r]
        rank_lens.append(first[0] if first is not None else 1)
    pairs = slots // 2

    kT = np.swapaxes(k, 1, 2)
    parts = np.arange(KT)

    in_maps = []
    slot_map = []
    for c in range(N_CORES):
        core_map = {}
        smap = []
        for j in range(pairs):
            qo, ko, width = pair_layout(rank_lens, j)
            na = rank_lens[2 * j]
            pkj = np.zeros((128, width), dtype=np.float32)
            pvj = np.zeros(
                (128, (na + rank_lens[2 * j + 1]) * (D + 1)), dtype=np.float16
            )
            for i, s in enumerate((2 * j, 2 * j + 1)):
                nr = rank_lens[s]
                seg = segs[N_CORES * s + c]
                if seg is None:
                    smap.append(None)
                    continue
                sz, b, qh, k0 = seg
                pb = i * 64
                smap.append((b, qh, k0))
                pkj[pb : pb + 64, qo : qo + QS] = q[b, qh * QS : (qh + 1) * QS, :].T
                kw = min(nr * KT, LK - k0 * KT)
                pkj[pb : pb + 64, ko : ko + kw] = kT[b, :, k0 * KT : k0 * KT + kw]
                voff = i * na * (D + 1)
                nv = kw // KT
                vs32 = np.zeros((128, nr, D + 1), dtype=np.float32)
                vs32[:, :nv, :D] = (
                    v[b, k0 * KT : k0 * KT + nv * KT, :]
                    .reshape(nv, KT, D)
                    .transpose(1, 0, 2)
                )
                vs32[:, :, D] = 1.0
                kid = (k0 + np.arange(nr))[None, :] * KT + parts[:, None]
                dead = (kid >= vl[b]) | (kid >= (k0 + sz) * KT)
                vs32[dead] = 0.0
                for kt in range(nr):
                    if dve_unit(s, kt):
                        vs32[:, kt, :] *= 1.0 / SW_GAIN
                pvj[:, voff : voff + nr * (D + 1)] = vs32.reshape(
                    128, nr * (D + 1)
                ).astype(np.float16)
            core_map[f"pk{j}"] = pkj
            core_map[f"pv{j}"] = pvj
        in_maps.append(core_map)
        slot_map.append(smap)
    return rank_lens, in_maps, slot_map


def scatter_out(results, slot_map):
    num = {}
    for c in range(N_CORES):
        oc = results[c]["out"]
        for s, seg in enumerate(slot_map[c]):
            if seg is None:
                continue
            b, qh, _ = seg
            blk = oc[s * 128 : (s + 1) * 128, :].astype(np.float64)
            key = (b, qh)
            if key in num:
                num[key] += blk
            else:
                num[key] = blk
    out = np.empty((B, LQ, D), dtype=np.float32)
    for (b, qh), a in num.items():
        a4 = a.reshape(128, 4, 65)
        res = a4[:, :, :D] / a4[:, :, D : D + 1]  # [128q, 4qc, D]
        out[b, qh * QS : (qh + 1) * QS, :] = res.transpose(1, 0, 2).reshape(QS, D)
    return out


def kernel(queries, keys, values, valid_lens, _run=None):
    rank_lens, in_maps, slot_map = plan_and_pack(queries, keys, values, valid_lens)
    nc = build_bass(rank_lens)
    if _run is not None:
        results = _run(nc, in_maps)
    else:
        import time as _time

        last = None
        for attempt in range(4):
            try:
                results = bass_utils.run_bass_kernel_spmd(
                    nc, in_maps, core_ids=list(range(N_CORES))
                ).results
                break
            except Exception as e:  # noqa: BLE001
                last = e
                _time.sleep(45.0 * (attempt + 1))
        else:
            raise last
    return scatter_out(results, slot_map)
